# revision 1
# baseline (speedup 1.0000x reference)
"""Autoregressive LSTM decoder (B=128, S=128, F=512, H=1024), tensor-parallel
over the hidden dimension across 8 TRN2 cores, raw-bass hand-scheduled, with
per-step remote_dma_broadcast exchange of h/y chunks.

Sharding (transposed layouts: dim on partitions, batch on free axis):
  - core c owns hidden slice c (128 dims): gates chunk = rows
    {g*H + 128c .. +128} for g in (i,f,g,o) -> 4 m-tiles; all matmuls N=128.
  - gates GEMM/step: 12 k-tiles (8 h first, 4 x last) x 4 m = 48 matmuls.
  - fc: cores c and c+4 both compute x-tile (c%4) (fc rows [128*(c%4),+128));
    the pair redundancy keeps broadcast tiles at the required 128 partitions.
  - h chunk and y chunk [128,128] f16 broadcast to all 8 cores (XOR-relative
    dests, 16 DMA engines) into hbuf/xbuf slot <partition_id>, parity
    double-buffered by step.

Semaphores are 12-bit (wrap at 4096), so every counting sem is rotated:
R=8 pools for the +16/step sems (arrivals, send-complete, output DMA) and
parity pools for the per-engine ledgers.  Per-step ledger (t = step):
  mm[t%2]    +1 gates GEMM done, +1 fc GEMM done        (PE)
  act[t%2]   +1 each: sgi,sgf,tng,sgo,tnc,ytanh         (ACT)
  dve[t%2]   +1 each: c=f*c, i*g, c+=, h=o*tanh(c)      (DVE)
  h_arr[t%8] +2 per arrived h(t)-chunk broadcast (16 when all in)
  x_arr[t%8] +2 per arrived y(t)-chunk broadcast
  lsem_*[t%8] +16 when step t's broadcast fully sent (staging reuse guard)
  ydma[t%8]  +16 when step t's yt output DMA done (staging reuse guard)
"""
import sys
sys.path.insert(0, "/opt/trn_rl_repo")
import numpy as np

B, S, F, H = 128, 128, 512, 1024
NC = 8
KX = F // 128      # 4
KH = H // 128      # 8
KT = KX + KH       # 12
R = 8              # rotation depth for +16/step sems
COLT = False        # 4x column-tiled matmuls (concurrent LDWEIGHTS streams)

_TP_CACHE = {}


def build(nsteps=S, local=False):
    """local=True strips all remote broadcasts and cross-core waits (numerics
    become garbage past step 0) — for cost-model / comm-exposure A/B only."""
    key = ("nc", nsteps, local)
    if key in _TP_CACHE:
        return _TP_CACHE[key]
    import concourse.bacc as bacc
    import concourse.mybir as mybir
    from concourse.bass import ds

    f16, f32 = mybir.dt.float16, mybir.dt.float32
    nc = bacc.Bacc(num_devices=NC)

    wg = nc.dram_tensor("wg", [KT, 128, 512], f16, kind="ExternalInput")
    wf = nc.dram_tensor("wf", [KH, 128, 128], f16, kind="ExternalInput")
    bg = nc.dram_tensor("bg", [128, 4], f32, kind="ExternalInput")
    bf = nc.dram_tensor("bf", [128, 1], f32, kind="ExternalInput")
    h0t = nc.dram_tensor("h0t", [128, H], f16, kind="ExternalInput")
    x0t = nc.dram_tensor("x0t", [128, F], f16, kind="ExternalInput")
    c0t = nc.dram_tensor("c0t", [128, 128], f32, kind="ExternalInput")
    # Timing aid: for nsteps > S, yt stays S slots (written modulo S) so
    # kernels with different nsteps have identical I/O footprints while every
    # step still performs its output DMA.
    yt_slots = min(nsteps, S)
    yt = nc.dram_tensor("yt", [yt_slots, 128, 128], f16, kind="ExternalOutput")

    in_sem = nc.alloc_semaphore("in_sem")
    mm = [nc.alloc_semaphore(f"mm{p}") for p in range(2)]
    act = [nc.alloc_semaphore(f"act{p}") for p in range(2)]
    dve = [nc.alloc_semaphore(f"dve{p}") for p in range(2)]
    h_arr = [nc.alloc_semaphore(f"h_arr{r}") for r in range(R)]
    x_arr = [nc.alloc_semaphore(f"x_arr{r}") for r in range(R)]
    lsem_h = [nc.alloc_semaphore(f"lsem_h{r}") for r in range(R)]
    lsem_x = [nc.alloc_semaphore(f"lsem_x{r}") for r in range(R)]
    ydma = [nc.alloc_semaphore(f"ydma{r}") for r in range(R)]

    RD = [(0, k) for k in range(8)]
    Sig = mybir.ActivationFunctionType.Sigmoid
    Tanh = mybir.ActivationFunctionType.Tanh

    wg_sb = [nc.alloc_sbuf_tensor(f"wg{k}", [128, 512], f16) for k in range(KT)]
    wf_sb = [nc.alloc_sbuf_tensor(f"wf{k}", [128, 128], f16) for k in range(KH)]
    bg_sb = nc.alloc_sbuf_tensor("bg_sb", [128, 4], f32)
    bf_sb = nc.alloc_sbuf_tensor("bf_sb", [128, 1], f32)
    hbuf = [nc.alloc_sbuf_tensor(f"hbuf{p}", [128, H], f16) for p in range(2)]
    xbuf = [nc.alloc_sbuf_tensor(f"xbuf{p}", [128, NC * 128], f16) for p in range(2)]
    h_stage = [nc.alloc_sbuf_tensor(f"hstg{p}", [128, 128], f16) for p in range(2)]
    x_stage = [nc.alloc_sbuf_tensor(f"xstg{p}", [128, 128], f16) for p in range(2)]
    cT = [nc.alloc_sbuf_tensor(f"cT{p}", [128, 128], f32) for p in range(2)]
    sgi = nc.alloc_sbuf_tensor("sgi", [128, 128], f32)
    sgf = nc.alloc_sbuf_tensor("sgf", [128, 128], f32)
    tng = nc.alloc_sbuf_tensor("tng", [128, 128], f32)
    sgo = nc.alloc_sbuf_tensor("sgo", [128, 128], f32)
    tnc = nc.alloc_sbuf_tensor("tnc", [128, 128], f32)
    tmp = nc.alloc_sbuf_tensor("tmp", [128, 128], f32)
    psg = [nc.alloc_psum_tensor(f"psg{p}", [128, 512], f32) for p in range(2)]
    psy = [nc.alloc_psum_tensor(f"psy{p}", [128, 128], f32) for p in range(2)]

    # ---- initial loads (sync/HWDGE), overlap with the entry barrier ----
    n_loads = 0
    for k in range(KT):
        nc.sync.dma_start(wg_sb[k][:], wg[k]).then_inc(in_sem, 16)
        n_loads += 1
    for k in range(KH):
        nc.sync.dma_start(wf_sb[k][:], wf[k]).then_inc(in_sem, 16)
        n_loads += 1
    for dst, src in ((bg_sb, bg), (bf_sb, bf), (hbuf[0], h0t), (cT[0], c0t)):
        nc.sync.dma_start(dst[:], src[:]).then_inc(in_sem, 16)
        n_loads += 1
    nc.sync.dma_start(xbuf[0][:, 0:F], x0t[:]).then_inc(in_sem, 16)
    n_loads += 1
    IN_ALL = 16 * n_loads

    # every core in-kernel (sems zeroed) before any remote write can land
    nc.all_core_barrier()

    pid = nc.gpsimd.partition_id()
    pid_off = pid * 128

    nc.tensor.wait_ge(in_sem, IN_ALL)
    nc.vector.wait_ge(in_sem, IN_ALL)
    nc.scalar.wait_ge(in_sem, IN_ALL)

    for t in range(nsteps):
        par, nxt = t % 2, (t + 1) % 2
        rot = t % R
        hb, hb_n, xb = hbuf[par], hbuf[nxt], xbuf[par]
        c_in, c_out = cT[par], cT[nxt]
        last = t == nsteps - 1
        # sem bases for this step
        ab = 6 * (t // 2)            # act[par] value before this step's ACTs
        db = 4 * (t // 2)            # dve[par] value before this step's DVE ops
        mb = 2 * (t // 2)            # mm[par] value before this step's GEMMs

        # ---------------- PE: gates GEMM (h k-tiles, then x) ----------------
        if t > 0 and not local:
            nc.tensor.wait_ge(h_arr[(t - 1) % R], 16 * ((t - 1) // R + 1))
        if t >= 2:
            # psg[par] WAR vs step t-2's gate-ACT reads
            nc.tensor.wait_ge(act[par], ab - 2)
        def gates_mm(k, rhs, first, last_km):
            wt = wg_sb[k]
            for m in range(4):
                for cq in range(4):
                    if COLT:
                        mmi = nc.tensor.matmul(
                            psg[par][32 * cq:32 * (cq + 1), 128 * m:128 * (m + 1)],
                            wt[:, 128 * m + 32 * cq:128 * m + 32 * (cq + 1)],
                            rhs,
                            start=(first and m == 0),
                            stop=(last_km and m == 3),
                            tile_position=(0, 32 * cq),
                        )
                    else:
                        if cq:
                            continue
                        mmi = nc.tensor.matmul(
                            psg[par][:, 128 * m:128 * (m + 1)],
                            wt[:, 128 * m:128 * (m + 1)],
                            rhs,
                            start=(first and m == 0),
                            stop=(last_km and m == 3),
                        )
            return mmi

        for k in range(KH):
            mmi = gates_mm(KX + k, hb[:, 128 * k:128 * (k + 1)], k == 0, False)
        if t > 0 and not local:
            nc.tensor.wait_ge(x_arr[(t - 1) % R], 16 * ((t - 1) // R + 1))
        for k in range(KX):
            mmi = gates_mm(k, xb[:, 128 * k:128 * (k + 1)], False, k == KX - 1)
        mmi.then_inc(mm[par], 1)

        # ---------------- ACT: gate nonlinearities ----------------
        nc.scalar.wait_ge(mm[par], mb + 1)
        if t >= 1:
            # sgi/sgf/tng/sgo WAR vs step t-1's DVE reads
            nc.scalar.wait_ge(dve[nxt], 4 * ((t - 1) // 2) + 4)
        # sgf first: the cell-update chain (mul1) only needs sigmoid(f)
        nc.scalar.activation(sgf[:], psg[par][:, 128:256], Sig,
                             bias=bg_sb[:, 1:2]).then_inc(act[par], 1)
        nc.scalar.activation(sgi[:], psg[par][:, 0:128], Sig,
                             bias=bg_sb[:, 0:1]).then_inc(act[par], 1)
        nc.scalar.activation(tng[:], psg[par][:, 256:384], Tanh,
                             bias=bg_sb[:, 2:3]).then_inc(act[par], 1)
        nc.scalar.activation(sgo[:], psg[par][:, 384:512], Sig,
                             bias=bg_sb[:, 3:4]).then_inc(act[par], 1)

        # ---------------- DVE: cell update ----------------
        nc.vector.wait_ge(act[par], ab + 1)
        if t >= 1:
            # c_in RAW vs step t-1's add; tmp WAR vs step t-1's add
            nc.vector.wait_ge(dve[nxt], 4 * ((t - 1) // 2) + 3)
        nc.vector.tensor_mul(c_out[:], sgf[:], c_in[:]).then_inc(dve[par], 1)
        nc.vector.wait_ge(act[par], ab + 3)
        nc.vector.tensor_mul(tmp[:], sgi[:], tng[:]).then_inc(dve[par], 1)
        nc.vector.wait_ge(dve[par], db + 2)
        nc.vector.tensor_add(c_out[:], c_out[:], tmp[:]).then_inc(dve[par], 1)

        # ACT: tanh(c')
        nc.scalar.wait_ge(dve[par], db + 3)
        nc.scalar.activation(tnc[:], c_out[:], Tanh).then_inc(act[par], 1)

        # DVE: h = sigmoid(o) * tanh(c')  (f16, into broadcast staging)
        nc.vector.wait_ge(act[par], ab + 5)
        if t >= 2 and not local:
            nc.vector.wait_ge(lsem_h[(t - 2) % R], 16 * ((t - 2) // R + 1))
        nc.vector.tensor_mul(h_stage[par][:], sgo[:], tnc[:]).then_inc(dve[par], 1)

        # ---------------- gpsimd: broadcast h (and later y) ----------------
        if not local:
            nc.gpsimd.remote_dma_broadcast(
                hb_n[:, ds(pid_off, 128)], h_stage[par][:], h_arr[rot],
                lsem_h[rot], rdests=RD)
            if not last:
                nc.gpsimd.remote_dma_broadcast(
                    xbuf[nxt][:, ds(pid_off, 128)], x_stage[par][:], x_arr[rot],
                    lsem_x[rot], rdests=RD)
            nc.gpsimd.wait_ge(dve[par], db + 4)
            nc.gpsimd.trigger_dma(1)      # fires the h broadcast

        # ---------------- PE: fc GEMM on gathered h(t) ----------------
        if not local:
            nc.tensor.wait_ge(h_arr[rot], 16 * (t // R + 1))
        if t >= 2:
            # psy[par] WAR vs step t-2's ytanh read
            nc.tensor.wait_ge(act[par], ab)
        for k in range(KH):
            rhs = hb_n[:, 128 * k:128 * (k + 1)]
            if COLT:
                for cq in range(4):
                    mmi = nc.tensor.matmul(
                        psy[par][32 * cq:32 * (cq + 1), :],
                        wf_sb[k][:, 32 * cq:32 * (cq + 1)],
                        rhs,
                        start=(k == 0),
                        stop=(k == KH - 1),
                        tile_position=(0, 32 * cq),
                    )
            else:
                mmi = nc.tensor.matmul(
                    psy[par][:],
                    wf_sb[k][:],
                    rhs,
                    start=(k == 0),
                    stop=(k == KH - 1),
                )
        mmi.then_inc(mm[par], 1)

        # ---------------- ACT: y = tanh(fc + b) ----------------
        nc.scalar.wait_ge(mm[par], mb + 2)
        if t >= 2:
            if not local:
                nc.scalar.wait_ge(lsem_x[(t - 2) % R], 16 * ((t - 2) // R + 1))
            nc.scalar.wait_ge(ydma[(t - 2) % R], 16 * ((t - 2) // R + 1))
        nc.scalar.activation(x_stage[par][:], psy[par][:], Tanh,
                             bias=bf_sb[:, 0:1]).then_inc(act[par], 1)

        # sync: output DMA;  gpsimd: fire y broadcast
        nc.sync.wait_ge(act[par], ab + 6)
        nc.sync.dma_start(yt[t % yt_slots], x_stage[par][:]).then_inc(
            ydma[rot], 16)
        if not last and not local:
            nc.gpsimd.wait_ge(act[par], ab + 6)
            nc.gpsimd.trigger_dma(1)      # fires the y broadcast

    # quiesce: all arrivals + local sends observed before kernel exit
    def _count(n, r):
        return len(range(r, n, R))
    for r in range(R):
        if _count(nsteps, r):
            if not local:
                nc.gpsimd.wait_ge(h_arr[r], 16 * _count(nsteps, r))
                nc.gpsimd.wait_ge(lsem_h[r], 16 * _count(nsteps, r))
            nc.sync.wait_ge(ydma[r], 16 * _count(nsteps, r))
        if not local and _count(nsteps - 1, r):
            nc.gpsimd.wait_ge(x_arr[r], 16 * _count(nsteps - 1, r))
            nc.gpsimd.wait_ge(lsem_x[r], 16 * _count(nsteps - 1, r))

    nc.compile()
    _TP_CACHE[key] = nc
    return nc


def prep_inputs(input, h0, c0, W_ih, W_hh, b_ih, b_hh, fc_W, fc_b, nsteps=S):
    """Build per-core in_maps."""
    W4 = np.concatenate([np.asarray(W_ih, np.float32),
                         np.asarray(W_hh, np.float32)], axis=1)  # [4H, F+H]
    b4 = np.asarray(b_ih, np.float32) + np.asarray(b_hh, np.float32)
    fc_W = np.asarray(fc_W, np.float32)
    fc_b = np.asarray(fc_b, np.float32)
    h0 = np.asarray(h0, np.float32)
    c0 = np.asarray(c0, np.float32)
    x0 = np.asarray(input, np.float32)[:, 0, :]

    # hbuf layout: [p, 128j + b] = h[b, 128j + p]
    h0t = np.ascontiguousarray(
        h0.reshape(B, KH, 128).transpose(2, 1, 0).reshape(128, H)
    ).astype(np.float16)
    x0t = np.ascontiguousarray(
        x0.reshape(B, KX, 128).transpose(2, 1, 0).reshape(128, F)
    ).astype(np.float16)

    in_maps = []
    for c in range(NC):
        Wc = np.stack([W4[g * H + 128 * c: g * H + 128 * (c + 1), :]
                       for g in range(4)])                     # [4,128m,1536]
        wg_np = np.ascontiguousarray(
            Wc.reshape(4, 128, KT, 128).transpose(2, 3, 0, 1).reshape(KT, 128, 512)
        ).astype(np.float16)
        cf = c % 4
        Fc = fc_W[128 * cf:128 * (cf + 1), :]                  # [128m,1024]
        wf_np = np.ascontiguousarray(
            Fc.reshape(128, KH, 128).transpose(1, 2, 0)
        ).astype(np.float16)
        bg_np = np.ascontiguousarray(
            np.stack([b4[g * H + 128 * c: g * H + 128 * (c + 1)]
                      for g in range(4)], axis=1)).astype(np.float32)
        bf_np = np.ascontiguousarray(
            fc_b[128 * cf:128 * (cf + 1)][:, None]).astype(np.float32)
        c0t_np = np.ascontiguousarray(
            c0[:, 128 * c:128 * (c + 1)].T).astype(np.float32)
        in_maps.append({
            "wg": wg_np, "wf": wf_np, "bg": bg_np, "bf": bf_np,
            "h0t": h0t, "x0t": x0t, "c0t": c0t_np,
        })
    return in_maps


def assemble_output(results, nsteps=S):
    """results[c]["yt"]: [min(nsteps,S), 128, 128] f16; x-tile j from core j."""
    n = min(nsteps, S)
    out = np.empty((B, n, F), np.float32)
    for j in range(4):
        ytv = np.asarray(results[j]["yt"], np.float32)  # [t, p, b]
        out[:, :, 128 * j:128 * (j + 1)] = ytv.transpose(2, 0, 1)
    return out


def _tp_kernel(input, h0, c0, W_ih, W_hh, b_ih, b_hh, fc_W, fc_b):
    from concourse.bass_utils import run_bass_kernel_spmd

    nc = build(S)
    in_maps = prep_inputs(input, h0, c0, W_ih, W_hh, b_ih, b_hh, fc_W, fc_b, S)
    res = run_bass_kernel_spmd(nc, in_maps, list(range(NC)))
    return assemble_output(res.results, S)


# ======================================================================
# Fallback: proven batch-parallel baseline (used if the TP path fails)
# ======================================================================
_BL_DOC = """Baseline: batch-parallel LSTM decoder (B=128, S=128, F=512, H=1024) on 8 TRN2 cores.

Strategy: data-parallel over batch (16 samples/core), weights replicated.
All state is kept TRANSPOSED ([dim, batch]) so that
  - gates GEMM: out[gate_dim, batch] = W_tile.T-stationary @ xh-tile streaming,
    with the gate dim on PSUM partitions -> full-width elementwise ops,
  - the fc output y^T is directly the next step's x^T lhsT tiles (zero
    on-device transposes; all layout marshalling happens on the host).
Weights/activations enter the PE in fp16 (fp32 accumulation in PSUM); the
cell state c stays fp32. Measured end-to-end error vs the fp32 reference
is ~5e-4 absmax (the LSTM recurrence damps per-step rounding).
"""
NCORES = 8
BL = B // NCORES          # 16 samples per core
KX = F // 128             # 4  x k-tiles
KH = H // 128             # 8  h k-tiles
KT = KX + KH              # 12 gates k-tiles
MG = 4 * H // 128         # 32 gates m-tiles
MF = F // 128             # 4  fc m-tiles
NSTEPS = S
# Timing aid: run NSTEPS steps but only DMA out the last OUT_STEPS of them,
# so kernels with different NSTEPS have identical I/O footprints.
OUT_STEPS = None  # None -> NSTEPS
# Split each [128K,128M] weight tile into 4 column sub-tiles loaded into
# independent 32-col PE array groups (concurrent LDWEIGHTS streams).
COL_TILING = False

_BL_CACHE = {}


def _split_sync_waits(nc, mybir, limit=1):
    """This toolchain's walrus accepts at most one semaphore wait per
    instruction; move the excess onto preceding same-engine NOPs."""
    cur_insts = nc.cur_bb.bb.instructions
    for f in nc.m.functions:
        for blk in f.blocks:
            insts = blk.instructions
            i = 0
            while i < len(insts):
                inst = insts[i]
                si = inst.sync_info
                if si and si.on_wait and len(si.on_wait) > limit:
                    waits = list(si.on_wait)
                    overflow, keep = waits[:-limit], waits[-limit:]
                    n_nops = 0
                    for j in range(0, len(overflow), limit):
                        chunk = overflow[j:j + limit]
                        nc.engines[inst.engine].nop(nofuse=True)
                        tail = cur_insts.pop()
                        assert "NoOp" in type(tail).__name__, type(tail).__name__
                        tail.sync_info = mybir.SyncInfo(on_wait=list(chunk), on_update=[])
                        insts.insert(i + n_nops, tail)
                        n_nops += 1
                    i += n_nops
                    inst.sync_info = mybir.SyncInfo(on_wait=keep, on_update=list(si.on_update))
                i += 1


def _bl_build():
    if "nc" in _BL_CACHE:
        return _BL_CACHE["nc"]
    import concourse.bass as bass
    import concourse.mybir as mybir
    import concourse.tile as tile

    f16, f32 = mybir.dt.float16, mybir.dt.float32
    nc = bass.Bass()

    wg = nc.dram_tensor("wg", [KT, 128, 4 * H], f16, kind="ExternalInput")
    wf = nc.dram_tensor("wf", [KH, 128, F], f16, kind="ExternalInput")
    bg = nc.dram_tensor("bg", [128, MG * BL], f32, kind="ExternalInput")
    bf = nc.dram_tensor("bf", [128, MF * BL], f32, kind="ExternalInput")
    x0 = nc.dram_tensor("x0", [128, KX * BL], f16, kind="ExternalInput")
    h0 = nc.dram_tensor("h0", [128, KH * BL], f16, kind="ExternalInput")
    c0 = nc.dram_tensor("c0", [128, KH * BL], f32, kind="ExternalInput")
    out_steps = OUT_STEPS or NSTEPS
    yt = nc.dram_tensor("yt", [out_steps, 128, MF * BL], f32, kind="ExternalOutput")

    with tile.TileContext(nc) as tc:
        with (
            tc.tile_pool(name="wpool", bufs=1) as wpool,
            tc.tile_pool(name="state", bufs=3) as st,
            tc.tile_pool(name="work", bufs=2) as wk,
            tc.tile_pool(name="psum", bufs=2, space="PSUM") as pp,
        ):
            wg_sb = []
            for k in range(KT):
                t = wpool.tile([128, 4 * H], f16, tag=f"wg{k}")
                nc.sync.dma_start(t[:], wg[k])
                wg_sb.append(t)
            wf_sb = []
            for k in range(KH):
                t = wpool.tile([128, F], f16, tag=f"wf{k}")
                nc.sync.dma_start(t[:], wf[k])
                wf_sb.append(t)
            bg_sb = wpool.tile([128, MG * BL], f32, tag="bg")
            nc.sync.dma_start(bg_sb[:], bg[:])
            bf_sb = wpool.tile([128, MF * BL], f32, tag="bf")
            nc.sync.dma_start(bf_sb[:], bf[:])

            xT = st.tile([128, KX * BL], f16, tag="xT")
            nc.sync.dma_start(xT[:], x0[:])
            hT = st.tile([128, KH * BL], f16, tag="hT")
            nc.sync.dma_start(hT[:], h0[:])
            cT = st.tile([128, KH * BL], f32, tag="cT")
            nc.sync.dma_start(cT[:], c0[:])

            Sig = mybir.ActivationFunctionType.Sigmoid
            Tanh = mybir.ActivationFunctionType.Tanh
            # k issue order: h-dependent tiles first so the PE works on them
            # while the fc->tanh->x chain of this step boundary completes.
            korder = list(range(KX, KT)) + list(range(KX))

            for t in range(NSTEPS):
                psg = pp.tile([128, MG * BL], f32, tag="psg")
                for ki, k in enumerate(korder):
                    if k < KX:
                        rhs = xT[:, BL * k:BL * (k + 1)]
                    else:
                        rhs = hT[:, BL * (k - KX):BL * (k - KX + 1)]
                    for m in range(MG):
                        # One accumulation group per PSUM bank: start zeroes the
                        # whole 2KB zero-region, so only the first matmul into
                        # the bank may carry start=True.
                        if COL_TILING:
                            for cq in range(4):
                                nc.tensor.matmul(
                                    psg[32 * cq:32 * (cq + 1), BL * m:BL * (m + 1)],
                                    wg_sb[k][:, 128 * m + 32 * cq:128 * m + 32 * (cq + 1)],
                                    rhs,
                                    start=(ki == 0 and m == 0),
                                    stop=(ki == KT - 1 and m == MG - 1),
                                    tile_position=(0, 32 * cq),
                                )
                        else:
                            nc.tensor.matmul(
                                psg[:, BL * m:BL * (m + 1)],
                                wg_sb[k][:, 128 * m:128 * (m + 1)],
                                rhs,
                                start=(ki == 0 and m == 0),
                                stop=(ki == KT - 1 and m == MG - 1),
                            )

                # layout: cols [0:128]=i, [128:256]=f, [256:384]=g, [384:512]=o
                # Bias-add in two halves so the first ACT starts earlier.
                gsb = wk.tile([128, MG * BL], f32, tag="gsb")
                nc.vector.tensor_add(gsb[:, 0:256], psg[:, 0:256], bg_sb[:, 0:256])
                sif = wk.tile([128, 256], f32, tag="sif")
                nc.scalar.activation(sif[:], gsb[:, 0:256], Sig)
                # c2 = sigmoid(f)*c as soon as sif lands
                cT2 = st.tile([128, KH * BL], f32, tag="cT")
                nc.vector.tensor_mul(cT2[:], sif[:, 128:256], cT[:])
                nc.vector.tensor_add(gsb[:, 256:512], psg[:, 256:512], bg_sb[:, 256:512])
                tng = wk.tile([128, 128], f32, tag="tng")
                nc.scalar.activation(tng[:], gsb[:, 256:384], Tanh)
                sgo = wk.tile([128, 128], f32, tag="sgo")
                nc.scalar.activation(sgo[:], gsb[:, 384:512], Sig)

                tmp = wk.tile([128, 128], f32, tag="tmp")
                nc.vector.tensor_mul(tmp[:], sif[:, 0:128], tng[:])
                nc.vector.tensor_add(cT2[:], cT2[:], tmp[:])
                # tanh(c) -> h in half-width chunks: the first 64 cols of hT2
                # land one ACT op earlier, releasing the fc GEMM's first
                # k-tiles sooner (cost-model verified: -119 ns/step).
                tnc = wk.tile([128, 128], f32, tag="tnc")
                hT2 = st.tile([128, KH * BL], f16, tag="hT")
                nc.scalar.activation(tnc[:, 0:64], cT2[:, 0:64], Tanh)
                nc.vector.tensor_mul(hT2[:, 0:64], sgo[:, 0:64], tnc[:, 0:64])
                nc.scalar.activation(tnc[:, 64:128], cT2[:, 64:128], Tanh)
                nc.vector.tensor_mul(hT2[:, 64:128], sgo[:, 64:128], tnc[:, 64:128])

                psy = pp.tile([128, MF * BL], f32, tag="psy")
                for ki in range(KH):
                    for m in range(MF):
                        if COL_TILING:
                            for cq in range(4):
                                nc.tensor.matmul(
                                    psy[32 * cq:32 * (cq + 1), BL * m:BL * (m + 1)],
                                    wf_sb[ki][:, 128 * m + 32 * cq:128 * m + 32 * (cq + 1)],
                                    hT2[:, BL * ki:BL * (ki + 1)],
                                    start=(ki == 0 and m == 0),
                                    stop=(ki == KH - 1 and m == MF - 1),
                                    tile_position=(0, 32 * cq),
                                )
                        else:
                            nc.tensor.matmul(
                                psy[:, BL * m:BL * (m + 1)],
                                wf_sb[ki][:, 128 * m:128 * (m + 1)],
                                hT2[:, BL * ki:BL * (ki + 1)],
                                start=(ki == 0 and m == 0),
                                stop=(ki == KH - 1 and m == MF - 1),
                            )
                ysb = wk.tile([128, MF * BL], f32, tag="ysb")
                nc.vector.tensor_add(ysb[:], psy[:], bf_sb[:])
                yout = wk.tile([128, MF * BL], f32, tag="yout")
                nc.scalar.activation(yout[:], ysb[:], Tanh)
                xT2 = st.tile([128, KX * BL], f16, tag="xT")
                nc.scalar.activation(xT2[:], ysb[:], Tanh)
                if t >= NSTEPS - out_steps:
                    nc.sync.dma_start(yt[t - (NSTEPS - out_steps)], yout[:])

                xT, hT, cT = xT2, hT2, cT2

    _split_sync_waits(nc, mybir, 1)
    _BL_CACHE["nc"] = nc
    return nc


def _bl_kernel(input, h0, c0, W_ih, W_hh, b_ih, b_hh, fc_W, fc_b):
    from concourse.bass_utils import run_bass_kernel_spmd

    nc = _bl_build()

    input = np.asarray(input, np.float32)
    h0 = np.asarray(h0, np.float32)
    c0 = np.asarray(c0, np.float32)
    W4 = np.concatenate([np.asarray(W_ih, np.float32),
                         np.asarray(W_hh, np.float32)], axis=1)  # [4H, F+H]
    # wg[k, p, j] = W4[j, 128k+p]
    wg_np = np.ascontiguousarray(
        W4.T.reshape(KT, 128, 4 * H).astype(np.float16))
    wf_np = np.ascontiguousarray(
        np.asarray(fc_W, np.float32).T.reshape(KH, 128, F).astype(np.float16))
    b4 = (np.asarray(b_ih, np.float32) + np.asarray(b_hh, np.float32))
    bg_np = np.ascontiguousarray(
        np.repeat(b4.reshape(MG, 128).T[:, :, None], BL, axis=2).reshape(128, MG * BL)
    ).astype(np.float32)
    bf_np = np.ascontiguousarray(
        np.repeat(np.asarray(fc_b, np.float32).reshape(MF, 128).T[:, :, None],
                  BL, axis=2).reshape(128, MF * BL)).astype(np.float32)

    def tconv(a, kt, dt):
        # a: [BL, kt*128] -> [128, kt*BL] with col 16k+b = a[b, 128k+p]
        return np.ascontiguousarray(
            a.T.reshape(kt, 128, BL).transpose(1, 0, 2).reshape(128, kt * BL)
        ).astype(dt)

    in_maps = []
    for c in range(NCORES):
        b0 = c * BL
        in_maps.append({
            "wg": wg_np, "wf": wf_np, "bg": bg_np, "bf": bf_np,
            "x0": tconv(input[b0:b0 + BL, 0, :], KX, np.float16),
            "h0": tconv(h0[b0:b0 + BL], KH, np.float16),
            "c0": tconv(c0[b0:b0 + BL], KH, np.float32),
        })

    res = run_bass_kernel_spmd(nc, in_maps, list(range(NCORES)))

    out = np.empty((B, S, F), np.float32)
    for c in range(NCORES):
        ytv = res.results[c]["yt"]  # [S, 128, MF*BL]
        # yt[t, p, 16m+b] = y_t[b, 128m+p]
        out[c * BL:(c + 1) * BL] = (
            ytv.reshape(S, 128, MF, BL).transpose(3, 0, 2, 1).reshape(BL, S, F)
        )
    return out


_LAST_PATH = None


def kernel(input, h0, c0, W_ih, W_hh, b_ih, b_hh, fc_W, fc_b):
    """Try the tensor-parallel remote-DMA kernel; fall back to the
    batch-parallel baseline on any failure (device-state robustness)."""
    global _LAST_PATH
    try:
        out = _tp_kernel(input, h0, c0, W_ih, W_hh, b_ih, b_hh, fc_W, fc_b)
        _LAST_PATH = "tp"
        return out
    except Exception as e:
        import traceback
        print("TP kernel path failed (%s); falling back to baseline" % e)
        traceback.print_exc()
        _LAST_PATH = "baseline"
        return _bl_kernel(input, h0, c0, W_ih, W_hh, b_ih, b_hh, fc_W, fc_b)

